# revision 53
# baseline (speedup 1.0000x reference)
"""Multi-head attention (B=2, T=2048, D=OUT=1024, H=16) on 8 TRN2 NeuronCores.

Sharding: data-parallel over batch (2 groups) x tensor-parallel over heads
(4 groups of 4 heads). Core c handles batch c//4, heads (c%4)*4..(c%4)*4+4.

Structure (transposed-PV, ~201us vs the 266us padded-PV baseline):
- Projections: Q^T/K^T tiles ([pair-depth, T], keys zero-padded per parity),
  V blocks with keys on partitions; bv enters the V psum chain as one K=1
  matmul (ones ⊗ bv_row) so evacuation is a single DVE op.
- S = kth^T qt per (head, key-tile): [128 keys, 1024 q] psum; exp on ACT with
  scale only -- the additive position bias (per key) is folded into V and the
  denominator column as exp(bias) (softmax identity: exp(s+b) = exp(s)e^b),
  which removes the bias operand from the activation (1114 vs 1336 ns).
- PV is TRANSPOSED: stationary = pt[:, qb*128:+128] (exp scores), moving =
  V' tile [128 keys, 65] (64 V cols scaled by e^bias + one e^bias column that
  accumulates the softmax denominator). Output psum [128 q, 65] per
  (head, q-block) chains over 16 key tiles: 65 moving cols per contraction
  pass instead of 1024, halving PV matmul time vs the padded layout.
- Normalize is per-partition and batched 4 q-blocks at a time: one strided
  reciprocal of four denominator columns + one stride-0-broadcast
  tensor_tensor multiply. No partition broadcasts or DMA hops.
- A PE transpose (identity moving, ~56ns) flips each normalized [128 q,
  128 c] head-pair block into the [c, T] layout the output projection
  consumes; its psum target is a filler-pool tile bitcast to bf16.
- Output projection: full (both-j) 256-contraction sums for tb0/1, j-split
  for tb2/3 whose j=1 half trails the final head (host sums those partials).
- Engine discipline: ACT runs ONLY the 128 exps (the kernel's pacer at
  ~1.11us each) + tail copies; every PSUM-reading op is on DVE (GPSIMD
  cannot access PSUM); GPSIMD gets SBUF-only work and low-priority DMA
  issues. PSUM banks: 4 (S, 2x[128,1024]) + 3 (filler pool) + 1 (PV slots).
- Input DMA: queues are round-robined by the 16 DMA engines, so priority =
  queue assignment: wk/wq on scalar, x nb0 as 8 per-kt chunks then nb1-3 as
  single issues on sync, packed consts + wv + wo on gpsimd. Tiny tensors are
  packed into one [128,402] f32 input (8-byte-row DMAs hog a queue ~6us).
- Emission: 128 S-unit slots with a priority-sorted plan: PV chains of the
  previous head at +2..+5 and +8,+10,+12,+14 with the batched normalizes
  given 2-slot runway before their psum slots are reused; transposes on odd
  slots after the pair's second-head normalize; projection groups placed
  against their S-deadlines; out-projection spread over slots 79-127 with 6
  jobs reserved to keep the tail dense.
"""

import numpy as np

import concourse.bass as bass
import concourse.mybir as mybir
import concourse.tile as tile
from concourse import bacc
from concourse.bass_utils import run_bass_kernel_spmd

B, T, D, OUT, H = 2, 2048, 1024, 1024, 16
DO = 256            # output columns per core (4 heads x 64)
DEPTH = 64
NH = 4              # heads per core
KT = D // 128       # 8 contraction tiles for the projections
TT = T // 128       # 16 key tiles
NB = T // 512       # 4 query/time blocks
F32 = mybir.dt.float32
F16 = mybir.dt.float16
BF16 = mybir.dt.bfloat16
MMDT = BF16
EXP = mybir.ActivationFunctionType.Exp
MULT = mybir.AluOpType.mult
ADD = mybir.AluOpType.add

H_ORDER = [1, 0, 2, 3]   # emission order of heads within a query-block pass
POS = H_ORDER + H_ORDER  # head by position p (0..7); qbp = p//4

_CACHE = {}


def build_attention(nc, dbg=False):
    if dbg:
        d_qt = [nc.declare_dram_parameter(f"d_qt{mi}", [128, T], MMDT, isOutput=True) for mi in range(2)]
        d_kt = [nc.declare_dram_parameter(f"d_kt{h}", [128, T], MMDT, isOutput=True) for h in range(NH)]
        d_vp = nc.declare_dram_parameter("d_vp", [128, TT, NH * 65], MMDT, isOutput=True)
        d_at = [nc.declare_dram_parameter(f"d_at{j}", [128, T], MMDT, isOutput=True) for j in range(2)]
        d_a2 = [nc.declare_dram_parameter(f"d_a2_{qbp}_{j}", [128, 8 * 128], MMDT, isOutput=True)
                for qbp in range(2) for j in range(2)]
        d_pt = nc.declare_dram_parameter("d_pt", [128, 1024], MMDT, isOutput=True)
    xt = nc.declare_dram_parameter("xt", [NB * D, 512], MMDT, isOutput=False)
    wq = nc.declare_dram_parameter("wq", [128, KT * DO], MMDT, isOutput=False)
    wk = nc.declare_dram_parameter("wk", [128, KT * DO], MMDT, isOutput=False)
    wv = nc.declare_dram_parameter("wv", [128, KT * DO], MMDT, isOutput=False)
    wo = nc.declare_dram_parameter("wo", [128, 2 * OUT], MMDT, isOutput=False)
    outT = nc.declare_dram_parameter("outT", [2 * NB * OUT, 512], F16, isOutput=True)

    with tile.TileContext(nc) as tc:
        with (
            tc.tile_pool(name="cw", bufs=1) as cw,
            tc.tile_pool(name="stage", bufs=8) as stage,
            tc.tile_pool(name="persist", bufs=1) as persist,
            tc.tile_pool(name="small", bufs=8) as small,
            tc.tile_pool(name="ptp", bufs=32) as ptp,
            tc.tile_pool(name="px", bufs=1) as px,
            tc.tile_pool(name="ps_s", bufs=2, space="PSUM") as ps_s,
            tc.tile_pool(name="ps_f", bufs=3, space="PSUM") as ps_f,
            tc.tile_pool(name="ps_pv", bufs=1, space="PSUM") as ps_pv,
        ):
            # ---- warmup: wake the PE HAM clock gate and the ACT exp table
            # while the input DMAs are in flight ----
            warm_w = cw.tile([128, 256], MMDT, tag="warmw")
            nc.gpsimd.memset(warm_w[:], 0.0)
            ones_f = cw.tile([128, NH], F32, tag="ones")
            nc.vector.memset(ones_f[:], 1.0)
            warm_ps = ps_s.tile([128, 256], F32, tag="s", name="warm_ps")
            for _ in range(26):
                nc.tensor.matmul(warm_ps[:, :256], warm_w[:, :128], warm_w[:, :256],
                                 start=True, stop=True)
            warm_pt = cw.tile([128, NH], MMDT, tag="warmpt")
            nc.scalar.activation(warm_pt[:], ones_f[:], EXP, scale=1.0)

            # ---- inputs ----
            # queue split (16 DMA engines round-robin across queues/DMAs):
            #  scalar: wk, wq (idle before the first exp; arrive ~9us)
            #  sync:   x nb0 as 8 per-kt chunks (first projections stream
            #          against arrival), then nb1/nb2/nb3 as one issue each
            #  gpsimd: all small constants packed into ONE [128,402] f32
            #          tensor (tiny row transfers would hog a queue for ~6us
            #          each), then wv, then wo (needed last)
            consts = nc.declare_dram_parameter("consts", [128, 402], F32, isOutput=False)
            consts_sb = cw.tile([128, 402], F32, tag="consts")
            nc.gpsimd.dma_start(out=consts_sb[:], in_=consts[:, :])
            bq_sb = consts_sb[:, 0:2]
            bv_sb = consts_sb[:, 2:258]
            eb_sb = consts_sb[:, 258:274]
            id_sb = cw.tile([128, 128], MMDT, tag="id")
            nc.vector.tensor_copy(out=id_sb[:], in_=consts_sb[:, 274:402])
            ones_row = cw.tile([1, 128], MMDT, tag="ones_row")
            nc.vector.memset(ones_row[:], 1.0)
            bv_row = cw.tile([1, DO], MMDT, tag="bv_row")
            nc.vector.tensor_copy(out=bv_row[:], in_=consts_sb[0:1, 2:258])

            def load_bf16(dram_ap, shape, tag, eng):
                r = px.tile(shape, MMDT, tag=tag, name=f"r_{tag}")
                eng.dma_start(out=r[:], in_=dram_ap)
                return r

            wk_r = load_bf16(wk[:, :].rearrange("p (kt m) -> p kt m", m=DO),
                             [128, KT, DO], "wk", nc.scalar)
            wq_r = load_bf16(wq[:, :].rearrange("p (kt m) -> p kt m", m=DO),
                             [128, KT, DO], "wq", nc.scalar)

            x_all = px.tile([128, KT, T], MMDT, tag="xall")
            xt_ap = xt.ap()
            for kt in range(KT):
                src = bass.AP(tensor=xt_ap.tensor,
                              offset=xt_ap.offset + kt * 128 * 512,
                              ap=[[512, 128], [1, 512]])
                nc.sync.dma_start(out=x_all[:, kt, 0:512], in_=src)
            for nb in range(1, NB):
                src = bass.AP(tensor=xt_ap.tensor, offset=xt_ap.offset + nb * D * 512,
                              ap=[[512, 128], [128 * 512, KT], [1, 512]])
                nc.sync.dma_start(out=x_all[:, :, nb * 512:(nb + 1) * 512], in_=src)
            wv_r = load_bf16(wv[:, :].rearrange("p (kt m) -> p kt m", m=DO),
                             [128, KT, DO], "wv", nc.gpsimd)
            wo_r = load_bf16(wo[:, :].rearrange("p (j n) -> p j n", j=2),
                             [128, 2, OUT], "wo", nc.gpsimd)

            # ---- persistent tiles ----
            qt2 = [persist.tile([128, T], MMDT, tag=f"qt{mi}", name=f"qt{mi}") for mi in range(2)]
            kth = [persist.tile([128, T], MMDT, tag=f"kh{h}", name=f"kh{h}") for h in range(NH)]
            vp = persist.tile([128, TT, NH * 65], MMDT, tag="vp")
            at2p = [persist.tile([128, T], MMDT, tag=f"atp{j}", name=f"atp{j}") for j in range(2)]
            attn2 = [[persist.tile([128, 8 * 128], MMDT, tag=f"a2_{qbp}_{j}",
                                   name=f"a2_{qbp}_{j}") for j in range(2)] for qbp in range(2)]
            for h in H_ORDER:
                lo, hi = ((64, 128) if h % 2 == 0 else (0, 64))
                nc.vector.memset(kth[h][lo:hi, :], 0.0)
            # e^bias columns of the V' tiles, all (tt, h) in one strided copy
            eba = eb_sb[:]
            eb_bcast = bass.AP(tensor=eba.tensor, offset=eba.offset,
                               ap=[eba.ap[0], eba.ap[1], [0, NH]])
            vp_cols = vp[:, :, :].rearrange("p t (h c) -> p t h c", c=65)[:, :, :, 64]
            nc.gpsimd.tensor_copy(out=vp_cols, in_=eb_bcast)

            # ---- helpers ----
            halves_open = {}

            def qk_part(which, mi, nb, part):
                key = (which, mi, nb)
                if part == 0:
                    halves_open[key] = ps_f.tile(
                        [128, 512], F32, tag="f", name=f"ps_{which}{mi}_{nb}")
                ps = halves_open[key]
                w_r = wq_r if which == "q" else wk_r
                for kt in range(part * 4, part * 4 + 4):
                    nc.tensor.matmul(
                        ps[:, :],
                        w_r[:, kt, mi * 128:(mi + 1) * 128],
                        x_all[:, kt, nb * 512:(nb + 1) * 512],
                        start=(kt == 0),
                        stop=(kt == KT - 1),
                    )
                if part == 1:
                    del halves_open[key]
                    sl = slice(nb * 512, (nb + 1) * 512)
                    if which == "q":
                        nc.vector.tensor_scalar_add(
                            qt2[mi][:, sl], ps[:, :], bq_sb[:, mi:mi + 1])
                    else:
                        nc.vector.tensor_scalar_add(
                            kth[2 * mi][0:64, sl], ps[0:64, :], bq_sb[0:64, mi:mi + 1])
                        nc.vector.tensor_scalar_add(
                            kth[2 * mi + 1][64:128, sl], ps[64:128, :],
                            bq_sb[64:128, mi:mi + 1])

            def q_group(mi, nb):
                qk_part("q", mi, nb, 0)
                qk_part("q", mi, nb, 1)

            def k_group(mi, nb):
                qk_part("k", mi, nb, 0)
                qk_part("k", mi, nb, 1)

            def v_group(tt):
                # x^T Wv chain plus one K=1 matmul adding bv (ones ⊗ bv_row),
                # so the evacuation is a single DVE op (x e^bias, psum->vp)
                ps = ps_f.tile([128, 512], F32, tag="f", name=f"ps_v{tt}")
                for kt in range(KT):
                    nc.tensor.matmul(
                        ps[:, :DO],
                        x_all[:, kt, tt * 128:(tt + 1) * 128],
                        wv_r[:, kt, :],
                        start=(kt == 0),
                        stop=False,
                    )
                nc.tensor.matmul(ps[:, :DO], ones_row[:], bv_row[:],
                                 start=False, stop=True)
                vpt = vp[:, tt, :].rearrange("p (h c) -> p h c", c=65)
                nc.vector.tensor_scalar_mul(
                    vpt[:, :, 0:64],
                    ps[:, :DO].rearrange("p (h c) -> p h c", c=64),
                    eb_sb[:, tt:tt + 1])

            pv_ps = ps_pv.tile([128, 4, 65], F32, tag="pv", name="pv_ps")
            pt_tiles = {}
            ccount = [0]

            def pv_norm(qbp, h, lo):
                # batched normalize of chains lo..lo+3: one reciprocal of the
                # four denominator columns + one stride-0-broadcast multiply
                rec4 = small.tile([128, 4], F32, tag="rec", name=f"rec{qbp}_{h}_{lo}")
                nc.vector.reciprocal_approx_fast(rec4[:], pv_ps[:, 0:4, 64])
                j, par = h // 2, h % 2
                r4 = rec4[:]
                rb = bass.AP(tensor=r4.tensor, offset=r4.offset,
                             ap=[r4.ap[0], [1, 4], [0, 64]])
                a2v = attn2[qbp][j][:, :].rearrange("p (qq c) -> p qq c", c=128)
                nc.vector.tensor_tensor(
                    a2v[:, lo:lo + 4, par * 64:par * 64 + 64],
                    pv_ps[:, 0:4, 0:64], rb, MULT)

            def pv_chain(qbp, h, qb):
                slot = qb % 4
                for kt in range(TT):
                    nc.tensor.matmul(
                        pv_ps[:, slot, :],
                        pt_tiles[(qbp, h, kt)][:, qb * 128:(qb + 1) * 128],
                        vp[:, kt, h * 65:(h + 1) * 65],
                        start=(kt == 0),
                        stop=(kt == TT - 1),
                    )
                # the second batch's normalize must be emitted before the next
                # head's chains reuse the psum slots; the first batch (lo=0) is
                # scheduled as its own plan item two slots after chain qb3
                if qb == 7:
                    pv_norm(qbp, h, 4)

            def transp(qbp, j, qb):
                # transpose psum target from the shared filler pool (bitcast
                # to bf16); no dedicated PSUM bank needed
                t = ps_f.tile([128, 512], F32, tag="f", name=f"tp{qbp}_{j}_{qb}")
                tv = t[:, 0:64].bitcast(MMDT)
                nc.tensor.transpose(tv, attn2[qbp][j][:, qb * 128:(qb + 1) * 128], id_sb[:])
                nc.vector.tensor_copy(
                    out=at2p[j][:, qbp * 1024 + qb * 128:qbp * 1024 + (qb + 1) * 128],
                    in_=tv)

            def c_half(j, nt, tb, evac=None):
                ps = ps_f.tile([128, 512], F32, tag="f", name=f"ps_c{j}_{nt}_{tb}")
                if j == "full":
                    for jj in range(2):
                        nc.tensor.matmul(
                            ps[:],
                            wo_r[:, jj, nt * 128:(nt + 1) * 128],
                            at2p[jj][:, tb * 512:(tb + 1) * 512],
                            start=(jj == 0),
                            stop=(jj == 1),
                        )
                    j = 0
                else:
                    nc.tensor.matmul(
                        ps[:],
                        wo_r[:, j, nt * 128:(nt + 1) * 128],
                        at2p[j][:, tb * 512:(tb + 1) * 512],
                        start=True,
                        stop=True,
                    )
                o_sb = stage.tile([128, 512], F16, tag="stage", name="o_sb")
                if evac == "s":
                    nc.scalar.copy(o_sb[:], ps[:])
                else:
                    nc.vector.tensor_copy(out=o_sb[:], in_=ps[:])
                nc.sync.dma_start(
                    out=outT[(j * NB + tb) * OUT + nt * 128:(j * NB + tb) * OUT + (nt + 1) * 128, :],
                    in_=o_sb[:],
                )

            # ---- prep: everything the first S unit needs, plus early V ----
            k_group(0, 0)
            q_group(0, 0)
            q_group(0, 1)
            v_group(0)
            v_group(1)
            k_group(0, 1)
            # V2/V3 move to plan slots 4/5: S(0) does not need them (their
            # consumers are the PV chains at slot ~19) and dropping them from
            # prep starts the exp stream ~3us earlier

            # ---- slot plan ----
            # items carry a priority so psum-freeing filler work (qk/v/c)
            # enqueues its DVE evacuation ahead of the chain recips within a
            # slot: prio 0 = qk/v/c fillers, 1 = PV chains, 2 = transposes
            plan = [[] for _ in range(128)]

            def put(s, fn, prio=0):
                plan[s].append((prio, len(plan[s]), fn))

            # v_groups: ALL must be complete (incl DVE add) before the
            # first PV chains at slot ~19
            vslots = [4, 5, 0, 1, 2, 3, 6, 7, 10, 11, 12, 13, 14, 15]
            for s, tt in zip(vslots, range(2, TT)):
                put(s, lambda tt=tt: v_group(tt))
            # K(0,2)/(0,3) just before the S units that need them
            put(4, lambda: qk_part("k", 0, 2, 0))
            put(5, lambda: qk_part("k", 0, 2, 1))
            put(8, lambda: qk_part("k", 0, 3, 0))
            put(9, lambda: qk_part("k", 0, 3, 1))
            # PV chains of the previous head: qb0-3 packed early, the first
            # normalize batch two slots later, qb4-7 spaced 2 apart so the DVE
            # normalizes always have runway before their psum slots are reused
            for p in range(1, 8):
                off = 1 if p == 1 else 0
                qbp_c, h_c = (p - 1) // 4, POS[p - 1]
                for qb in range(4):
                    put(16 * p + 2 + off + qb,
                        lambda qb=qb, q_=qbp_c, h_=h_c: pv_chain(q_, h_, qb), prio=1)
                put(16 * p + 6 + off, lambda q_=qbp_c, h_=h_c: pv_norm(q_, h_, 0), prio=1)
                for k, qb in enumerate(range(4, 8)):
                    put(16 * p + 8 + off + 2 * k,
                        lambda qb=qb, q_=qbp_c, h_=h_c: pv_chain(q_, h_, qb), prio=1)
            # transposes after the pair's second-head normalize batches:
            # qb0-3 interleave the odd slots behind the norm batch, qb4-7 land
            # at the start of the following group
            for sp, qbp, j in [(1, 0, 0), (3, 0, 1), (5, 1, 0)]:
                for qb in range(4):
                    put(16 * (sp + 1) + 7 + 2 * qb,
                        lambda qbp=qbp, j=j, qb=qb: transp(qbp, j, qb), prio=2)
                for qb in range(4, 8):
                    put(16 * (sp + 2) + (qb - 4),
                        lambda qbp=qbp, j=j, qb=qb: transp(qbp, j, qb), prio=2)
            # projection groups against their S deadlines (hand layout)
            qk_sched = [
                (16, "q", 1, 0, 0), (17, "q", 1, 0, 1),
                (26, "q", 1, 1, 0), (27, "q", 1, 1, 1),
                (28, "k", 1, 0, 0), (29, "k", 1, 0, 1),
                (30, "k", 1, 1, 0), (31, "k", 1, 1, 1),
                (32, "k", 1, 2, 0), (33, "k", 1, 2, 1),
                (34, "k", 1, 3, 0), (35, "k", 1, 3, 1),
                (44, "q", 0, 2, 0), (45, "q", 0, 2, 1),
                (46, "q", 0, 3, 0), (47, "q", 0, 3, 1),
                (48, "q", 1, 2, 0), (49, "q", 1, 2, 1),
                (52, "q", 1, 3, 0), (53, "q", 1, 3, 1),
            ]
            for s, w, m, n, part in qk_sched:
                put(s, lambda w=w, m=m, n=n, p=part: qk_part(w, m, n, p))
            # output projection jobs, greedy from their ready slots: full
            # (both-j) sums for tb0/1, j-split for tb2/3 (j=1 trails in the tail)
            # the last 6 j0/tb3 jobs are RESERVED for the tail: they are the
            # only independent PE work left there, and interleaving them with
            # the chain->transpose->c(j1) serial stream keeps the PE dense
            # (and the HAM clock up) through the tail
            c_jobs = [(79, "full", 0), (85, "full", 1), (111, 0, 2), (117, 0, 3)]
            tail_c = []
            for ready, j, tb in c_jobs:
                s = ready
                for nt in range(OUT // 128):
                    if j == 0 and tb == 3:
                        tail_c.append((j, nt, tb))
                        continue
                    while s < 128 and len(plan[s]) >= 2:
                        s += 1
                    if s >= 128:
                        raise RuntimeError("c jobs did not fit")
                    put(s, lambda j=j, nt=nt, tb=tb: c_half(j, nt, tb), prio=0)
                    s += 1

            # ---- main stream: 128 S units ----
            # S matmuls are emitted ONE SLOT AHEAD of their slot's other work:
            # the exp stream on ACT is the kernel's pacer, and the early S
            # keeps a ready psum input queued for it so PE jitter in the
            # fillers/chains never bubbles the ACT queue.
            units = [(p // 4, POS[p], kt) for p in range(8) for kt in range(TT)]
            s_tiles = {}

            def s_unit(idx):
                qbp, h, kt = units[idx]
                s_ps = ps_s.tile([128, 1024], F32, tag="s", name=f"s_{idx}")
                for half in range(2):
                    nc.tensor.matmul(
                        s_ps[:, half * 512:(half + 1) * 512],
                        kth[h][:, kt * 128:(kt + 1) * 128],
                        qt2[h // 2][:, qbp * 1024 + half * 512:qbp * 1024 + (half + 1) * 512],
                        start=True,
                        stop=True,
                    )
                s_tiles[idx] = s_ps

            for idx, (qbp, h, kt) in enumerate(units):
                s_unit(idx)
                pt = ptp.tile([128, 1024], MMDT, tag="pt", name=f"pt{idx}")
                nc.scalar.activation(pt[:], s_tiles.pop(idx)[:], EXP, scale=0.125)
                pt_tiles[(qbp, h, kt)] = pt
                if dbg and idx == 0:
                    nc.sync.dma_start(out=d_pt[:, :], in_=pt[:])
                for _, _, item in sorted(plan[idx]):
                    item()

            # ---- tail: last head's PV, j=1 transposes, j=1 out-proj tb2/3 ----
            last_h = POS[7]
            tc_iter = iter(tail_c)

            def tail_fill():
                nxt = next(tc_iter, None)
                if nxt is not None:
                    j, nt, tb = nxt
                    c_half(j, nt, tb, evac="s")

            for qb in range(4):
                pv_chain(1, last_h, qb)
                tail_fill()
            pv_norm(1, last_h, 0)
            for qb in range(4, 8):
                pv_chain(1, last_h, qb)
                transp(1, 1, qb - 4)
                tail_fill()
            transp(1, 1, 3)
            for nt in range(0, 4):
                c_half(1, nt, 2, evac=("s" if nt % 2 else None))
            for qb in range(4, 8):
                transp(1, 1, qb)
            for nt in range(4, 8):
                c_half(1, nt, 2, evac=("s" if nt % 2 else None))
            for nt in range(8):
                c_half(1, nt, 3, evac=("s" if nt % 2 else None))

            if dbg:
                for mi in range(2):
                    nc.sync.dma_start(out=d_qt[mi][:, :], in_=qt2[mi][:])
                for h in range(NH):
                    nc.sync.dma_start(out=d_kt[h][:, :], in_=kth[h][:])
                nc.sync.dma_start(out=d_vp[:, :, :], in_=vp[:])
                for j in range(2):
                    nc.sync.dma_start(out=d_at[j][:, :], in_=at2p[j][:])
                for qbp in range(2):
                    for j in range(2):
                        nc.sync.dma_start(out=d_a2[qbp * 2 + j][:, :], in_=attn2[qbp][j][:])



def _build():
    nc = bacc.Bacc(trn_type="TRN2")
    build_attention(nc)
    nc.compile()
    return nc


def _get_nc():
    if "nc" not in _CACHE:
        _CACHE["nc"] = _build()
    return _CACHE["nc"]


def make_in_maps(x, W_q, b_q, W_k, W_v, b_v, W_o, bias):
    import ml_dtypes
    bf16 = ml_dtypes.bfloat16

    def warr(w):
        # [D, DO] -> SBUF layout [128, KT*DO] (partition-major, kt-tiled)
        return np.ascontiguousarray(
            w.reshape(KT, 128, DO).transpose(1, 0, 2).reshape(128, KT * DO))

    def woarr(w):
        # [2*128, OUT] -> [two*64+p, j, n] -> [128, 2*OUT]
        return np.ascontiguousarray(
            w.reshape(2, 2, 64, OUT).transpose(1, 2, 0, 3).reshape(128, 2 * OUT))

    in_maps = []
    xtb = [np.ascontiguousarray(
        x[b].T.astype(bf16).reshape(D, NB, 512).transpose(1, 0, 2).reshape(NB * D, 512))
        for b in range(B)]
    wqb = W_q.astype(bf16)
    wkb = W_k.astype(bf16)
    wvb = W_v.astype(bf16)
    wob = W_o.astype(bf16)
    ebias = np.ascontiguousarray(np.exp(bias.astype(np.float64)).astype(np.float32)
                                 .reshape(TT, 128).T)
    ident = np.eye(128, dtype=np.float32)
    for c in range(8):
        b, hg = divmod(c, 4)
        sl = slice(hg * DO, (hg + 1) * DO)
        consts = np.empty((128, 402), dtype=np.float32)
        consts[:, 0:2] = b_q[sl].reshape(2, 128).T
        consts[:, 2:258] = np.broadcast_to(b_v[sl], (128, DO))
        consts[:, 258:274] = ebias
        consts[:, 274:402] = ident
        in_maps.append({
            "xt": xtb[b],
            "wq": warr(wqb[:, sl]),
            "wk": warr(wkb[:, sl]),
            "wv": warr(wvb[:, sl]),
            "wo": woarr(wob[sl, :]),
            "consts": np.ascontiguousarray(consts),
        })
    return in_maps


def kernel(x, W_q, b_q, W_k, b_k, W_v, b_v, W_o, b_o, bias, **_ignored):
    x = np.asarray(x, dtype=np.float32)
    W_q = np.asarray(W_q, dtype=np.float32)
    W_k = np.asarray(W_k, dtype=np.float32)
    W_v = np.asarray(W_v, dtype=np.float32)
    W_o = np.asarray(W_o, dtype=np.float32)
    b_q = np.asarray(b_q, dtype=np.float32)
    b_v = np.asarray(b_v, dtype=np.float32)
    b_o = np.asarray(b_o, dtype=np.float32)
    bias = np.asarray(bias, dtype=np.float32)

    nc = _get_nc()
    in_maps = make_in_maps(x, W_q, b_q, W_k, W_v, b_v, W_o, bias)
    _CACHE["in_maps"] = in_maps
    res = run_bass_kernel_spmd(nc, in_maps, list(range(8)))
    out = np.zeros((B, T, OUT), dtype=np.float32)
    for c in range(8):
        oc = res.results[c]["outT"].reshape(2, NB, OUT, 512).astype(np.float32)
        parts = [oc[0, 0], oc[0, 1], oc[0, 2] + oc[1, 2], oc[0, 3] + oc[1, 3]]
        out[c // 4] += np.concatenate(parts, axis=1).T
    out += b_o
    return out
